# revision 24
# baseline (speedup 1.0000x reference)
"""GQA attention-with-KV-cache kernel for Trainium2, sharded over 8 NeuronCores.

Problem: B=32, Q=16 new tokens, DIM=4096, 32 Q-heads / 8 KV-heads, head_dim=128,
cache len 4096 (16 appended at start_pos=4080), rotary on q/k, causal mask.

Sharding: tensor-parallel over KV heads - core c owns KV head c and Q heads
4c..4c+3. Each core computes its heads' attention plus the partial out @ wo_shard;
the host sums the 8 partial outputs (the TP all-reduce).

Device program (per core), designed to stay at the HBM-DMA roofline:
  phase 1: xq = x @ wq (PSUM-accumulated over 32 dim-chunks), wide-op rotary,
           DMA-transpose to q^T tiles.
  phase 2 (per group of 2 batches): scores are computed TRANSPOSED -
           matmul(lhsT=k-chunk [d,128s], rhs=q^T [d,64]) -> s^T [128s, 64q'] in
           PSUM - so the exp activation writes p^T straight to SBUF with no
           transpose, no normalize pass, and no cross-window reduction. The
           softmax denominator comes free from a ones-column appended to each
           V chunk (out free dim 129); normalization is a per-partition scalar
           on the small o tile [64, 128]. One PE transpose [128,128] per group
           puts o^T into attnT for the wo matmul, which is interleaved across
           groups. KV tiles are prefetched 2 groups ahead.

Host-side prep (input marshalling): shard/cast/transpose weights and cache to
bf16 DMA-friendly layouts, compute the 16 appended k/v rows (x @ wk/wv + rotary,
0.5 GFLOP) and splice them into the cache shards.
"""
import sys
sys.path.insert(0, "/opt/trn_rl_repo")

import numpy as np
import ml_dtypes
from contextlib import ExitStack

import concourse.bass as bass
import concourse.bacc as bacc
import concourse.tile as tile
import concourse.mybir as mybir

BF16 = ml_dtypes.bfloat16

B, Q, DIM = 32, 16, 4096
NH, NKV, HD = 32, 8, 128
NREP = NH // NKV          # 4 q-heads per kv-head
S = 4096                  # cache length
START = S - Q             # 4080
NT = B * Q                # 512 tokens
P = 128
NCORES = 8
QP = NREP * Q             # 64 = q' rows per batch (4 heads x 16 tokens)
NC = S // P               # 32 seq chunks of 128
VW = HD + 1               # v chunk width incl. ones column (129)

_CACHE = {}
MARKS = []                 # (label, instruction-id watermark) filled by _build_nc


def _build_nc(debug=False):
    """Build the single-core Bass program (same program on all 8 cores; only the
    data differs per core)."""
    nc = bacc.Bacc("TRN2", target_bir_lowering=False, debug=debug, num_devices=NCORES)
    dt = mybir.dt
    f32 = dt.float32
    bf16 = dt.bfloat16

    # ---- DRAM I/O (per-core shard layouts, prepared on host) ----
    xT_d = nc.dram_tensor("xT", (32, P, NT), bf16, kind="ExternalInput")       # x^T tiles [dim-chunk][128, 512tok]
    wq_d = nc.dram_tensor("wq_sh", (32, P, NREP * HD), bf16, kind="ExternalInput")
    wo_d = nc.dram_tensor("wo_sh", (4, P, DIM), bf16, kind="ExternalInput")    # [c-chunk][128, 4096]
    kT_d = nc.dram_tensor("kT", (B, P, S), bf16, kind="ExternalInput")         # per b: updated keys^T [d, seq]
    v_d = nc.dram_tensor("vp", (B, P, NC * VW), bf16, kind="ExternalInput")    # per b: [p][c*129+d] = v[c*128+p, d]; d=128 -> 1.0
    cosq_d = nc.dram_tensor("cosq", (P, 4 * (HD // 2)), f32, kind="ExternalInput")  # q rotary, 1/sqrt(HD)-scaled, tiled x4
    sinq_d = nc.dram_tensor("sinq", (P, 4 * (HD // 2)), f32, kind="ExternalInput")
    # additive causal mask for the 16 appended positions, staged at partitions
    # 112..127 so the add is partition-aligned with the last score chunk
    maskT_d = nc.dram_tensor("maskT", (P, QP), f32, kind="ExternalInput")
    id_d = nc.dram_tensor("ident", (P, P), bf16, kind="ExternalInput")
    out_d = nc.dram_tensor("out_p", (NT, DIM), dt.float16, kind="ExternalOutput")  # partial output (pre all-reduce)

    with ExitStack() as ctx:
        tc = ctx.enter_context(tile.TileContext(nc))

        # ---------- persistent tiles ----------
        cpool = ctx.enter_context(tc.tile_pool(name="const", bufs=1))
        ident = cpool.tile([P, P], bf16, tag="ident")
        cos4 = cpool.tile([P, 4 * (HD // 2)], f32, tag="cos4")
        sin4 = cpool.tile([P, 4 * (HD // 2)], f32, tag="sin4")
        maskT = cpool.tile([P, QP], f32, tag="maskT")
        # [d, (b, hb, q)] rotated q^T, bf16 - one tile per token-chunk so early
        # attention groups only depend on their own chunk of phase 1
        qTb_t = [cpool.tile([P, 8 * QP], bf16, tag=f"qTb{t}", name=f"qTb{t}")
                 for t in range(4)]

        def qTb_sl(b):
            return qTb_t[b // 8][:, (b % 8) * QP:(b % 8 + 1) * QP]
        attnT = cpool.tile([P, 4 * NT], bf16, tag="attnT")    # [d, (hb, tok)] attention out^T

        nc.sync.dma_start(ident[:], id_d.ap())
        nc.sync.dma_start(cos4[:], cosq_d.ap())
        nc.sync.dma_start(sin4[:], sinq_d.ap())
        nc.sync.dma_start(maskT[:], maskT_d.ap())

        # kv prefetch pool (ctx-level so loads can be issued during phase 1).
        # Separate tags per stream so bufs=3 = prefetch depth 2 per stream.
        kpool = ctx.enter_context(tc.tile_pool(name="kv", bufs=3))
        kv_tiles = {}

        def emit_kv(g):
            if g >= B // 2:
                return
            b0, b1 = 2 * g, 2 * g + 1
            kt0 = kpool.tile([P, S], bf16, tag="kt0", name=f"kt{b0}")
            kt1 = kpool.tile([P, S], bf16, tag="kt1", name=f"kt{b1}")
            vt0 = kpool.tile([P, NC * VW], bf16, tag="vt0", name=f"vt{b0}")
            vt1 = kpool.tile([P, NC * VW], bf16, tag="vt1", name=f"vt{b1}")
            nc.sync.dma_start(kt0[:], kT_d.ap()[b0])
            nc.sync.dma_start(kt1[:], kT_d.ap()[b1])
            nc.sync.dma_start(vt0[:], v_d.ap()[b0])
            nc.sync.dma_start(vt1[:], v_d.ap()[b1])
            kv_tiles[g] = (kt0, kt1, vt0, vt1)

        # ---------- phase 1: xq projection + rotary + transpose ----------
        # dim-chunk-outer streaming: x^T / wq blocks of 8 dim-chunks are
        # consumed once each (bufs=4: all prefetched, so PE never gaps and
        # ramps to full clock), accumulating all 4 token-chunk PSUM tiles.
        with tc.tile_pool(name="ph1", bufs=4) as p1, \
             tc.tile_pool(name="ph1w", bufs=2) as p1w, \
             tc.tile_pool(name="ph1ps", bufs=1, space="PSUM") as p1ps:
            pq = [p1ps.tile([P, NREP * HD], f32, tag=f"pq{t}", name=f"pq{t}")
                  for t in range(4)]
            blk_tiles = []
            for blk in range(4):  # 1 MiB per DMA, 8 dim-chunks
                xb = p1.tile([P, 8 * NT], bf16, tag="xTblk", name=f"xTb{blk}")
                wb = p1.tile([P, 8 * 512], bf16, tag="wqblk", name=f"wqb{blk}")
                k0 = blk * 8
                nc.sync.dma_start(
                    xb[:].rearrange("p (a b) -> p a b", a=8),
                    xT_d.ap()[k0:k0 + 8].rearrange("a p b -> p a b"))
                nc.sync.dma_start(
                    wb[:].rearrange("p (a b) -> p a b", a=8),
                    wq_d.ap()[k0:k0 + 8].rearrange("a p b -> p a b"))
                blk_tiles.append((xb, wb))
            # kv for the first 2 groups behind the phase-1 loads (blocks 2/3
            # WAR-wait on blocks 0/1 at the queue head for only ~1us)
            emit_kv(0)
            emit_kv(1)

            for blk in range(4):
                xb, wb = blk_tiles[blk]
                for dkl in range(8):
                    for t in range(4):
                        lhs = xb[:, dkl * NT + t * P: dkl * NT + (t + 1) * P]
                        nc.tensor.matmul(pq[t][:], lhs,
                                         wb[:, dkl * 512:(dkl + 1) * 512],
                                         start=(blk == 0 and dkl == 0),
                                         stop=(blk == 3 and dkl == 7))

            for t in range(4):
                # rotary over all 4 heads at once (cos/sin pre-scaled by
                # 1/sqrt(HD), tiled x4); out bf16
                qrot = p1w.tile([P, NREP * HD], bf16, tag="qrot")
                e = pq[t][:, 0:512:2]
                o = pq[t][:, 1:512:2]
                t1 = p1w.tile([P, 256], f32, tag="t1")
                t2 = p1w.tile([P, 256], f32, tag="t2")
                nc.vector.tensor_mul(t1[:], e, cos4[:])
                nc.vector.tensor_mul(t2[:], o, sin4[:])
                nc.vector.tensor_sub(qrot[:, 0:512:2], t1[:], t2[:])
                t3 = p1w.tile([P, 256], f32, tag="t1")
                t4 = p1w.tile([P, 256], f32, tag="t2")
                nc.vector.tensor_mul(t3[:], e, sin4[:])
                nc.vector.tensor_mul(t4[:], o, cos4[:])
                nc.vector.tensor_add(qrot[:, 1:512:2], t3[:], t4[:])

                # transpose to [d, (hb, tok)] on PE (4 x 128x128 pieces into
                # PSUM; no DMA involved), then scatter to qTb
                qtp = p1ps.tile([P, NREP * HD], bf16, tag="qtp", bufs=2)
                for hb in range(NREP):
                    nc.tensor.transpose(qtp[:, hb * P:(hb + 1) * P],
                                        qrot[:, hb * P:(hb + 1) * P], ident[:])
                dst = qTb_t[t][:].rearrange("p (b hb q) -> p b hb q", hb=NREP, q=Q)
                src = qtp[:].rearrange("p (hb j q) -> p j hb q", hb=NREP, q=Q)
                nc.vector.tensor_copy(dst, src)

        # ---------- phase 2: attention over the cache, 2 batches per group ----------
        # wo weights live in space freed by the phase-1 pools (first needed at
        # group 4). The four 1-MiB chunk loads are interleaved into the SP
        # queue at the ends of groups 0/1 so the kv stream is never blocked
        # for long.
        wpool = ctx.enter_context(tc.tile_pool(name="wop", bufs=1))
        wo_sb = wpool.tile([P, 4 * DIM], bf16, tag="wo")      # [c-chunk d, (hb, outdim)]

        def emit_wo_load(hb):
            nc.sync.dma_start(wo_sb[:, hb * DIM:(hb + 1) * DIM], wo_d.ap()[hb])

        ppool = ctx.enter_context(tc.tile_pool(name="p", bufs=2))
        smallp = ctx.enter_context(tc.tile_pool(name="small", bufs=2))
        spool = ctx.enter_context(tc.tile_pool(name="spsum", bufs=2, space="PSUM"))
        opool = ctx.enter_context(tc.tile_pool(name="opsum", bufs=2, space="PSUM"))
        otpool = ctx.enter_context(tc.tile_pool(name="otpsum", bufs=2, space="PSUM"))
        wopool = ctx.enter_context(tc.tile_pool(name="wopsum", bufs=2, space="PSUM"))

        # wo work (tcT, od) spread over groups: token-chunk tcT completes at
        # group 4*tcT+3; emit 2 od-chunks per group from 4*tcT+4 on.
        wo_sched = {}
        for tcT in range(4):
            for j in range(4):
                g_at = 4 * tcT + 4 + j
                pairs = [(tcT, 2 * j), (tcT, 2 * j + 1)]
                if g_at < 16:
                    wo_sched.setdefault(g_at, []).extend(pairs)
                else:
                    wo_sched.setdefault(-1, []).extend(pairs)  # after last group

        def emit_wo(tcT, od):
            pw = wopool.tile([P, 512], f32, tag="pw", name="pw")
            for hb in range(4):
                nc.tensor.matmul(
                    pw[:],
                    attnT[:, hb * NT + tcT * P: hb * NT + (tcT + 1) * P],
                    wo_sb[:, hb * DIM + od * 512: hb * DIM + (od + 1) * 512],
                    start=(hb == 0), stop=(hb == 3))
            ow = ppool.tile([P, 512], dt.float16, tag="ow", name="ow")
            nc.vector.tensor_copy(ow[:], pw[:])
            nc.sync.dma_start(
                out_d.ap()[tcT * P:(tcT + 1) * P, od * 512:(od + 1) * 512],
                ow[:])

        MARKS.clear()
        MARKS.append(("phase2", nc.next_id()))
        for g in range(B // 2):
            MARKS.append((f"g{g}", nc.next_id()))
            b0, b1 = 2 * g, 2 * g + 1
            kt0, kt1, vt0, vt1 = kv_tiles.pop(g)

            # p^T tile [s-in-chunk, (b, c, q')]: batch-major, chunk-major
            pT = ppool.tile([P, 2 * NC * QP], bf16, tag="pT")

            # scores^T super-tiles (8 chunks each) -> exp -> pT, per batch
            for bi, (b, kt) in enumerate(((b0, kt0), (b1, kt1))):
                for cs in range(4):
                    ps = spool.tile([P, 8 * QP], f32, tag="s")
                    for k in range(8):
                        c = cs * 8 + k
                        nc.tensor.matmul(ps[:, k * QP:(k + 1) * QP],
                                         kt[:, c * P:(c + 1) * P], qTb_sl(b))
                    if cs == 3:
                        # causal mask on the 16 appended positions: chunk 31,
                        # s_local 112..127. Partition base must be 32-aligned,
                        # so add over 96:128 (rows 96..111 of maskT are zero).
                        nc.vector.tensor_add(ps[96:P, 7 * QP:8 * QP],
                                             ps[96:P, 7 * QP:8 * QP],
                                             maskT[96:P, :])
                    nc.scalar.activation(
                        pT[:, bi * NC * QP + cs * 8 * QP: bi * NC * QP + (cs + 1) * 8 * QP],
                        ps[:], mybir.ActivationFunctionType.Exp)

            emit_kv(g + 2)
            if g < 4:
                emit_wo_load(g)

            # wo chunks for already-completed token chunks (fills PE while the
            # last exps drain on ACT)
            for tcT, od in wo_sched.get(g, []):
                emit_wo(tcT, od)

            # p @ v_ext -> o_ext [q', 128+1], col 128 = softmax denominator.
            # Both batches land in one PSUM tile at partition offsets 0/64
            # (tile_position) so the normalize ops stay partition-aligned.
            onorm = ppool.tile([P, HD], bf16, tag="onorm")
            po = opool.tile([P, VW], f32, tag="po")
            for bi, (b, vt) in enumerate(((b0, vt0), (b1, vt1))):
                for c in range(NC):
                    nc.tensor.matmul(po[bi * QP:(bi + 1) * QP, :],
                                     pT[:, bi * NC * QP + c * QP: bi * NC * QP + (c + 1) * QP],
                                     vt[:, c * VW:(c + 1) * VW],
                                     start=(c == 0), stop=(c == NC - 1),
                                     tile_position=(0, bi * QP))
            rinv = smallp.tile([P, 1], f32, tag="rinv")
            nc.vector.reciprocal(rinv[:], po[:, HD:HD + 1])
            nc.vector.tensor_scalar_mul(onorm[:], po[:, 0:HD], rinv[:])

            # one PE transpose puts both batches' o into attnT layout
            oT = otpool.tile([P, P], bf16, tag="oT")
            nc.tensor.transpose(oT[:], onorm[:], ident[:])
            dst = attnT[:].rearrange("p (hb t) -> p hb t", hb=NREP)[
                :, :, b0 * Q:(b1 + 1) * Q].rearrange("p hb (b q) -> p hb b q", b=2)
            src = oT[:].rearrange("p (b hb q) -> p hb b q", b=2, hb=NREP)
            nc.vector.tensor_copy(dst, src)

        # trailing wo chunks (last token chunk)
        for tcT, od in wo_sched.get(-1, []):
            emit_wo(tcT, od)

    nc.compile()
    return nc


def _host_prep(x, cache_k, cache_v, freqs_cis, mask, wq, wk, wv, wo):
    """Build the 8 per-core input maps. Computes the 16 appended k/v rows here
    (cheap projection) and splices them into the cache shards."""
    xf = np.asarray(x, dtype=np.float32).reshape(NT, DIM)
    xbf = xf.astype(BF16).astype(np.float32)      # reference casts x to bf16 first
    xT = np.ascontiguousarray(xbf.T).astype(BF16).reshape(32, P, NT)

    wq = np.asarray(wq); wk = np.asarray(wk); wv = np.asarray(wv); wo = np.asarray(wo)

    fc = np.asarray(freqs_cis)
    if np.iscomplexobj(fc):
        cos16 = np.real(fc).astype(np.float32)    # (16, 64)
        sin16 = np.imag(fc).astype(np.float32)
    else:
        cos16 = np.cos(fc).astype(np.float32)
        sin16 = np.sin(fc).astype(np.float32)
    scale = np.float32(1.0 / np.sqrt(HD))
    cosq = np.tile(cos16, (8, 4)) * scale         # (128, 256) rows: q = r % 16
    sinq = np.tile(sin16, (8, 4)) * scale

    # appended k/v rows (host projection, matches reference numerics closely:
    # bf16-valued operands, fp32 accumulate)
    wkf = wk.astype(np.float32)
    wvf = wv.astype(np.float32)
    xk = (xbf @ wkf).reshape(B, Q, NKV, HD)
    xv = (xbf @ wvf).reshape(B, Q, NKV, HD)
    e = xk[..., 0::2]; o = xk[..., 1::2]
    c4 = cos16[None, :, None, :]; s4 = sin16[None, :, None, :]
    xkr = np.empty_like(xk)
    xkr[..., 0::2] = e * c4 - o * s4
    xkr[..., 1::2] = e * s4 + o * c4

    # full updated cache, then per-core layouts
    ck = np.asarray(cache_k, dtype=np.float32).copy()
    cv = np.asarray(cache_v, dtype=np.float32).copy()
    ck[:, START:S] = xkr
    cv[:, START:S] = xv

    kT_all = np.ascontiguousarray(ck.transpose(2, 0, 3, 1)).astype(BF16)  # (kv, b, d, s)
    # v ext layout per (kv, b): [p, c*129 + d] = v[c*128+p, d]; col 128 = 1.0
    v_r = cv.reshape(B, NC, P, NKV, HD).transpose(3, 0, 2, 1, 4)          # (kv, b, p, c, d)
    v_ext = np.empty((NKV, B, P, NC, VW), dtype=BF16)
    v_ext[..., :HD] = v_r.astype(BF16)
    v_ext[..., HD] = np.asarray(1.0, dtype=BF16)
    v_ext = v_ext.reshape(NKV, B, P, NC * VW)

    # mask^T for the appended tail at partitions 112..127:
    # maskT[112+sl, hb*16+q] = mask[q, 4080+sl]
    mask_np = np.asarray(mask, dtype=np.float32)
    maskT = np.zeros((P, QP), dtype=np.float32)
    maskT[P - Q:P] = np.tile(mask_np[:, START:S].T, (1, NREP))

    ident = np.eye(P, dtype=BF16)

    in_maps = []
    for c in range(NCORES):
        hq0 = c * NREP * HD
        in_maps.append({
            "xT": xT,
            "wq_sh": np.ascontiguousarray(wq[:, hq0:hq0 + NREP * HD]).astype(BF16).reshape(32, P, NREP * HD),
            "wo_sh": np.ascontiguousarray(wo[hq0:hq0 + NREP * HD, :]).astype(BF16).reshape(4, P, DIM),
            "kT": kT_all[c],
            "vp": v_ext[c],
            "cosq": cosq, "sinq": sinq,
            "maskT": maskT, "ident": ident,
        })
    return in_maps


def _get_nc():
    if "nc" not in _CACHE:
        _CACHE["nc"] = _build_nc(debug=False)
    return _CACHE["nc"]


def kernel(x, cache_k, cache_v, freqs_cis, mask, wq, wk, wv, wo, start_pos):
    assert int(start_pos) == START, f"kernel hardcodes start_pos={START}"
    from concourse import bass_utils
    nc = _get_nc()
    in_maps = _host_prep(x, cache_k, cache_v, freqs_cis, mask, wq, wk, wv, wo)
    res = bass_utils.run_bass_kernel_spmd(nc, in_maps, core_ids=list(range(NCORES)))
    out = np.zeros((NT, DIM), dtype=np.float32)
    for c in range(NCORES):
        out += np.asarray(res.results[c]["out_p"], dtype=np.float32)
    return out.reshape(B, Q, DIM)
